# revision 19
# baseline (speedup 1.0000x reference)
"""DGCNN semantic-segmentation kernel for 8x Trainium2 NeuronCores.

Strategy: data-parallel over batch. B=4 samples; core c processes sample c%4
end-to-end (cores 4-7 duplicate work so one SPMD program runs everywhere);
host takes outputs from cores 0-3. Everything for one sample stays on one
core: no cross-core communication.

Per-sample pipeline (all on device):
  3x EdgeConv blocks: kNN (fp16 hi/lo split K=13 distance matmul, ~1e-6
  accurate) -> top-20 selection (pack 8-bit chunk-local index into low
  mantissa bits, top-8 per 256-chunk via DVE max8, refine via
  max8/match_replace, recover columns arithmetically) -> streamed gather via
  GPSIMD ap_gather -> folded 1x1 conv + instance-norm + leaky-relu ->
  second conv (fp32 matmul) streamed with running max over k (max commutes
  with the monotone normalize+lrelu since scale > 0). Then the global head
  (w6..w9) with in1d norms, mostly bf16 matmuls.

Inter-block layout: packed [128, 2048] fp32 - partition p<64 holds channel p
for points n<2048, partition 64+p holds channel p for n>=2048.
"""

import sys

if "/opt/trn_rl_repo" not in sys.path:
    sys.path.insert(0, "/opt/trn_rl_repo")

import numpy as np

N = 4096
NT = 32          # row tiles of 128 for the distance/selection loop
HB = 2048
KNN = 20
NK = KNN * HB    # free size of the (virtual) packed h tensor per partition
CHUNK = 256
NEG = -3.0e38

_CACHE = {}


def _build_program():
    import concourse.bacc as bacc
    import concourse.tile as tile
    from concourse import mybir
    from contextlib import ExitStack

    F32 = mybir.dt.float32
    F16 = mybir.dt.float16
    BF16 = mybir.dt.bfloat16
    U32 = mybir.dt.uint32
    U16 = mybir.dt.uint16
    I16 = mybir.dt.int16
    AF = mybir.ActivationFunctionType
    ALU = mybir.AluOpType
    AX = mybir.AxisListType

    nc = bacc.Bacc("TRN2", target_bir_lowering=False, debug=False, num_devices=8)

    def din(name, shape, dt=F32):
        return nc.dram_tensor(name, shape, dt, kind="ExternalInput").ap()

    xt_d = din("xt", [4, N])
    waT_d = [din("waT1", [4, 64]), din("waT3", [64, 64]), din("waT5", [64, 64])]
    wbT_d = [din("wbT1", [4, 64]), din("wbT3", [64, 64]), din("wbT5", [64, 64])]
    wcT_d = [din("w2T", [64, 64]), din("w4T", [64, 64])]
    w6T_d = [din(f"w6T_{k}", [64, 1024], BF16) for k in range(3)]
    w7gT_d = [din(f"w7gT_{k}", [128, 512], BF16) for k in range(8)]
    w7xT_d = [din(f"w7xT_{k}", [64, 512], BF16) for k in range(3)]
    w8T_d = [din(f"w8T_{k}", [128, 256], BF16) for k in range(4)]
    w9T_d = [din(f"w9T_{k}", [128, 2], BF16) for k in range(2)]
    iota256_d = din("iota256", [128, CHUNK], U32)
    maskc_d = din("maskc", [128, 1], U32)
    magic_d = din("magic", [128, 1], U32)
    id2_d = din("id2", [2, 2])

    out_d = nc.dram_tensor("out", [N, 2], F32, kind="ExternalOutput").ap()

    with tile.TileContext(nc) as tc, ExitStack() as ctx:
        wpool = ctx.enter_context(tc.tile_pool(name="wpool", bufs=1))
        xpool = ctx.enter_context(tc.tile_pool(name="xpool", bufs=1))
        stpool = ctx.enter_context(tc.tile_pool(name="stpool", bufs=1))
        pp = ctx.enter_context(tc.tile_pool(name="pp", bufs=2, space="PSUM"))
        ectx = ExitStack()
        abpool = ectx.enter_context(tc.tile_pool(name="abpool", bufs=1))
        gpool = ectx.enter_context(tc.tile_pool(name="gpool", bufs=1))
        dpool = ectx.enter_context(tc.tile_pool(name="dpool", bufs=3))
        selpool = ectx.enter_context(tc.tile_pool(name="selpool", bufs=4))
        idxpool = ectx.enter_context(tc.tile_pool(name="idxpool", bufs=1))
        chpool = ectx.enter_context(tc.tile_pool(name="chpool", bufs=1))

        def load(pool, ap_d, shape, dt=F32, dup64=False):
            rows = shape[0]
            tshape = [128, shape[1]] if dup64 else shape
            t = pool.tile(tshape, dt, tag=ap_d.tensor.name, name=ap_d.tensor.name + "_sb")
            nc.sync.dma_start(t[0:rows, :], ap_d)
            if dup64:
                nc.sync.dma_start(t[64:64 + rows, :], ap_d)
            return t

        waT = [load(wpool, waT_d[0], [4, 64], dup64=True),
               load(wpool, waT_d[1], [64, 64], dup64=True),
               load(wpool, waT_d[2], [64, 64], dup64=True)]
        wbT = [load(wpool, wbT_d[0], [4, 64], dup64=True),
               load(wpool, wbT_d[1], [64, 64], dup64=True),
               load(wpool, wbT_d[2], [64, 64], dup64=True)]
        wcT = [load(wpool, wcT_d[0], [64, 64], dup64=True),
               load(wpool, wcT_d[1], [64, 64], dup64=True), None]
        w6T = [load(wpool, a, [64, 1024], BF16, dup64=True) for a in w6T_d]
        w7gT = [load(wpool, a, [128, 512], BF16) for a in w7gT_d]
        w7xT = [load(wpool, a, [64, 512], BF16, dup64=True) for a in w7xT_d]
        w8T = [load(wpool, a, [128, 256], BF16) for a in w8T_d]
        w9T = [load(wpool, a, [128, 2], BF16) for a in w9T_d]
        iota256 = load(wpool, iota256_d, [128, CHUNK], U32)
        maskc = load(wpool, maskc_d, [128, 1], U32)
        magic = load(wpool, magic_d, [128, 1], U32)
        id2 = load(wpool, id2_d, [2, 2])

        xt_sb = gpool.tile([4, N], F32, tag="ya_dup", name="xt_sb")
        nc.sync.dma_start(xt_sb[:], xt_d)
        x_p = [xpool.tile([128, HB], F32, tag=f"x{i}_p", name=f"x{i}_p") for i in range(4)]
        nc.sync.dma_start(x_p[0][0:4, :], xt_sb[:, 0:HB])
        nc.sync.dma_start(x_p[0][64:68, :], xt_sb[:, HB:N])

        def small(tag, shape=(128, 1), dt=F32):
            return stpool.tile(list(shape), dt, tag=tag, name=tag)

        def ts(out, in0, s1, op0, s2=None, op1=None):
            if op1 is None:
                nc.vector.tensor_scalar(out, in0, s1, None, op0)
            else:
                nc.vector.tensor_scalar(out, in0, s1, s2, op0, op1)

        def rsqrt_inplace(y, t_in, rows):
            r = small("rs_r", (rows, 1))
            nc.vector.reciprocal(r[:], t_in)
            nc.scalar.activation(y, r[:], AF.Sqrt)

        def scale_bias_from_mv(mv):
            """mv [128,2] per-partition (mean, var); rows p/p+64 are halves of one
            channel. Returns s128, b128 [128,1] with normalized = s*x + b."""
            mvb = small("st_mvb", (64, 2))
            nc.sync.dma_start(mvb[:], mv[64:128, :])
            m = small("st_m", (64, 1)); v = small("st_v", (64, 1))
            dm = small("st_dm", (64, 1))
            nc.vector.tensor_tensor(m[:], mv[0:64, 0:1], mvb[:, 0:1], ALU.add)
            ts(m[:], m[:], 0.5, ALU.mult)
            nc.vector.tensor_tensor(v[:], mv[0:64, 1:2], mvb[:, 1:2], ALU.add)
            nc.vector.tensor_tensor(dm[:], mv[0:64, 0:1], mvb[:, 0:1], ALU.subtract)
            nc.vector.tensor_tensor(dm[:], dm[:], dm[:], ALU.mult)
            ts(v[:], v[:], 0.5, ALU.mult)
            ts(dm[:], dm[:], 0.25, ALU.mult)
            nc.vector.tensor_tensor(v[:], v[:], dm[:], ALU.add)
            ts(v[:], v[:], 1e-5, ALU.add)
            s = small("st_s", (64, 1))
            rsqrt_inplace(s[:], v[:], 64)
            bb = small("st_bb", (64, 1))
            nc.vector.tensor_tensor(bb[:], m[:], s[:], ALU.mult)
            ts(bb[:], bb[:], -1.0, ALU.mult)
            sb128 = small("st_sb128", (128, 2))
            nc.vector.tensor_copy(sb128[0:64, 0:1], s[:])
            nc.vector.tensor_copy(sb128[0:64, 1:2], bb[:])
            nc.sync.dma_start(sb128[64:128, :], sb128[0:64, :])
            return sb128

        def mv_from_sums(ssum, ssq, count):
            """[128, w] partial sums -> mv [128, 2] (mean, var per partition)."""
            mv = small("sm_mv", (128, 2))
            nc.vector.tensor_reduce(mv[:, 0:1], ssum[:], axis=AX.X, op=ALU.add)
            nc.vector.tensor_reduce(mv[:, 1:2], ssq[:], axis=AX.X, op=ALU.add)
            ts(mv[:, 0:1], mv[:, 0:1], 1.0 / count, ALU.mult)
            ts(mv[:, 1:2], mv[:, 1:2], 1.0 / count, ALU.mult)
            m2 = small("sm_m2")
            nc.vector.tensor_tensor(m2[:], mv[:, 0:1], mv[:, 0:1], ALU.mult)
            nc.vector.tensor_tensor(mv[:, 1:2], mv[:, 1:2], m2[:], ALU.subtract)
            return mv

        # ---------------- EdgeConv block ----------------
        def edge_block(bi, xin_p, C):
            has_conv2 = bi < 2

            # distance operand prep: pieces computed at partition base 0 (ACT
            # alignment rule), assembled into aT/bT via DMAs.
            sq = dpool.tile([128, HB], F32, tag="dpk", name="sq")
            nc.scalar.activation(sq[0:3, :], xin_p[0:3, :], AF.Square)
            nc.scalar.activation(sq[64:67, :], xin_p[64:67, :], AF.Square)
            ones3 = abpool.tile([128, 1], F32, tag="ones3")
            nc.gpsimd.memset(ones3[:], 1.0)

            src4 = gpool.tile([4, N], F32, tag="ya_dup", name="src4")
            for h in range(2):
                psx = pp.tile([128, HB], F32, tag="pp")
                for j in range(4):
                    nc.tensor.matmul(
                        psx[0:1, 512 * j:512 * (j + 1)],
                        ones3[64 * h:64 * h + 3, 0:1],
                        sq[64 * h:64 * h + 3, 512 * j:512 * (j + 1)])
                nc.scalar.copy(src4[0:1, HB * h:HB * (h + 1)], psx[0:1, 0:HB])
                nc.sync.dma_start(src4[1:4, HB * h:HB * (h + 1)], xin_p[64 * h:64 * h + 3, :])

            hi4 = gpool.tile([4, N], F16, tag="M", name="hi4")
            lo4 = gpool.tile([4, N], F16, tag="yb_p", name="lo4")
            nc.scalar.copy(hi4[:], src4[:])
            nc.vector.scalar_tensor_tensor(
                lo4[:], hi4[:], -1.0, src4[:], ALU.mult, ALU.add)
            nh4 = dpool.tile([4, N], F16, tag="dpk", name="nh4")
            nl4 = dpool.tile([4, N], F16, tag="dpk", name="nl4")
            nc.scalar.mul(nh4[:], hi4[:], -1.0)
            nc.scalar.mul(nl4[:], lo4[:], -1.0)

            # aT rows: [1, 1, -xxh, -xxl, 2ph(3), 2ph(3), 2pl(3)]
            # bT rows: [-xxh, -xxl, 1, 1, ph(3), pl(3), ph(3)]
            aT = abpool.tile([16, N], F16, tag="aT")
            bT = abpool.tile([16, N], F16, tag="bT")
            nc.gpsimd.memset(aT[0:2, :], 1.0)
            nc.sync.dma_start(bT[2:4, :], aT[0:2, :])
            nc.sync.dma_start(aT[2:3, :], nh4[0:1, :])
            nc.sync.dma_start(aT[3:4, :], nl4[0:1, :])
            nc.sync.dma_start(bT[0:1, :], nh4[0:1, :])
            nc.sync.dma_start(bT[1:2, :], nl4[0:1, :])
            h2x = dpool.tile([4, N], F16, tag="dpk", name="h2x")
            l2x = dpool.tile([4, N], F16, tag="dpk", name="l2x")
            nc.scalar.mul(h2x[:], hi4[:], 2.0)
            nc.scalar.mul(l2x[:], lo4[:], 2.0)
            nc.sync.dma_start(aT[4:7, :], h2x[1:4, :])
            nc.sync.dma_start(aT[7:10, :], h2x[1:4, :])
            nc.sync.dma_start(aT[10:13, :], l2x[1:4, :])
            nc.sync.dma_start(bT[4:7, :], hi4[1:4, :])
            nc.sync.dma_start(bT[7:10, :], lo4[1:4, :])
            nc.sync.dma_start(bT[10:13, :], hi4[1:4, :])

            # ya (duplicated to both partition halves, full n) and yb (packed)
            ya_dup = gpool.tile([128, N], F32, tag="ya_dup")
            yb_p = gpool.tile([128, HB], F32, tag="yb_p")
            for dsth in range(2):
                po = 64 * dsth
                psy = pp.tile([128, HB], F32, tag="pp")
                for srch in range(2):
                    for j in range(4):
                        sl = slice(512 * j, 512 * (j + 1))
                        nc.tensor.matmul(
                            psy[po:po + 64, sl],
                            waT[bi][64 * srch:64 * srch + C, :],
                            xin_p[64 * srch:64 * srch + C, sl])
                    nc.scalar.copy(
                        ya_dup[po:po + 64, HB * srch:HB * (srch + 1)],
                        psy[po:po + 64, 0:HB])
                psb = pp.tile([128, HB], F32, tag="pp")
                for j in range(4):
                    sl = slice(512 * j, 512 * (j + 1))
                    nc.tensor.matmul(
                        psb[po:po + 64, sl],
                        wbT[bi][64 * dsth:64 * dsth + C, :],
                        xin_p[64 * dsth:64 * dsth + C, sl])
                nc.scalar.copy(
                    yb_p[po:po + 64, :].rearrange(
                        "p (g t q) -> p t g q", g=8, t=16, q=16),
                    psb[po:po + 64, 0:HB].rearrange(
                        "p (t g q) -> p t g q", t=16, g=8, q=16))

            # ---- distance + selection ----
            colbuf = idxpool.tile([128, 768], U16, tag="colbuf")
            posall = idxpool.tile([128, 768], U16, tag="posall")
            v24all = idxpool.tile([128, 768], F32, tag="v24all")
            for t in range(NT):
                lhs = aT[0:13, 128 * t:128 * (t + 1)]
                cand = selpool.tile([128, 128], F32, tag="cand")
                for h in range(2):
                    psd = pp.tile([128, HB], F32, tag="pp")
                    for j in range(4):
                        nc.tensor.matmul(
                            psd[:, 512 * j:512 * (j + 1)], lhs,
                            bT[0:13, HB * h + 512 * j:HB * h + 512 * (j + 1)])
                    dpk = dpool.tile([128, HB], U32, tag="dpk")
                    nc.vector.scalar_tensor_tensor(
                        dpk[:].rearrange("p (a c) -> p a c", c=CHUNK),
                        psd[:].bitcast(U32).rearrange("p (a c) -> p a c", c=CHUNK),
                        maskc[:, :],
                        iota256[:].rearrange("p (a c) -> p a c", a=1).broadcast_to([128, 8, CHUNK]),
                        ALU.bitwise_and, ALU.bitwise_or)
                    for c in range(8):
                        nc.vector.max(
                            cand[:, 64 * h + 8 * c:64 * h + 8 * (c + 1)],
                            dpk[:].bitcast(F32)[:, CHUNK * c:CHUNK * (c + 1)])
                v24 = v24all[:, 24 * t:24 * (t + 1)]
                pos = posall[:, 24 * t:24 * (t + 1)]
                c2 = selpool.tile([128, 128], F32, tag="c2")
                c3 = selpool.tile([128, 128], F32, tag="c3")
                nc.vector.max(v24[:, 0:8], cand[:])
                nc.vector.match_replace(c2[:], v24[:, 0:8], cand[:], NEG)
                nc.vector.max(v24[:, 8:16], c2[:])
                nc.vector.match_replace(c3[:], v24[:, 8:16], c2[:], NEG)
                nc.vector.max(v24[:, 16:24], c3[:])
                nc.vector.max_index(pos[:, 0:8], v24[:, 0:8], cand[:])
                nc.vector.max_index(pos[:, 8:16], v24[:, 8:16], cand[:])
                nc.vector.max_index(pos[:, 16:24], v24[:, 16:24], cand[:])
            # batched column arithmetic: col = (pos>>3)*256 + (v24.bits & 255)
            locb = idxpool.tile([128, 768], U32, tag="locb")
            ts(locb[:], v24all[:].bitcast(U32), 255, ALU.bitwise_and)
            loc16b = idxpool.tile([128, 768], U16, tag="loc16b")
            nc.vector.tensor_copy(loc16b[:], locb[:])
            ts(posall[:], posall[:], 3, ALU.logical_shift_right)
            ts(posall[:], posall[:], 8, ALU.logical_shift_left)
            nc.vector.tensor_tensor(
                colbuf[:].rearrange("p (j t) -> p t j", j=24),
                posall[:].rearrange("p (t j) -> p t j", j=24),
                loc16b[:].rearrange("p (t j) -> p t j", j=24), ALU.add)

            # ---- reformat into per-core wrapped gather index lists ----
            # list (per half): position i = j*2048 + n_loc; stored wrapped-16:
            # partition 16k + (i%16), free i//16 = j*128 + t_loc*8 + g2
            wrapped = idxpool.tile([128, 2560], U16, tag="wrapped")
            for h in range(2):
                for g2 in range(8):
                    src = colbuf[16 * g2:16 * (g2 + 1), :] \
                        .rearrange("p (j t) -> p j t", t=NT)[:, 0:KNN, 16 * h:16 * (h + 1)]
                    dst = wrapped[64 * h:64 * h + 16, :] \
                        .rearrange("p (j g t) -> p j g t", g=8, t=16)[:, :, g2, :]
                    nc.sync.dma_start(dst, src)
                for k in range(1, 4):
                    nc.sync.dma_start(
                        wrapped[64 * h + 16 * k:64 * h + 16 * (k + 1), :],
                        wrapped[64 * h:64 * h + 16, :])

            ya3 = ya_dup[:].rearrange("p (m d) -> p m d", d=1)
            wri = wrapped[:].bitcast(I16)

            # ---- pass 1: streamed gather (2 k per instr) -> bn stats ----
            M = gpool.tile([128, HB], F32, tag="M")
            nc.gpsimd.memset(M[:], NEG)
            h1sum = small("h1sum", (128, KNN // 2))
            h1sq = small("h1sq", (128, KNN))
            sscr1 = chpool.tile([128, HB], F32, tag="sscr", bufs=1, name="sscr1")
            yb_b2 = yb_p[:].rearrange("p (o n) -> p o n", o=1).broadcast_to([128, 2, HB])
            for q in range(KNN // 2):
                gch = chpool.tile([128, 2 * HB], F32, tag="gch", bufs=2)
                nc.gpsimd.ap_gather(
                    gch[:], ya3, wri[:, 256 * q:256 * (q + 1)],
                    channels=128, num_elems=N, d=1, num_idxs=2 * HB)
                nc.vector.scalar_tensor_tensor(
                    gch[:].rearrange("p (j n) -> p j n", j=2),
                    gch[:].rearrange("p (j n) -> p j n", j=2),
                    1.0, yb_b2, ALU.mult, ALU.add,
                    accum_out=h1sum[:, q:q + 1])
                for r in range(2):
                    nc.scalar.activation(
                        sscr1[:], gch[:, HB * r:HB * (r + 1)], AF.Square,
                        accum_out=h1sq[:, 2 * q + r:2 * q + r + 1])
                    if not has_conv2:
                        nc.vector.tensor_tensor(
                            M[:], M[:], gch[:, HB * r:HB * (r + 1)], ALU.max)
            mv1 = mv_from_sums(h1sum, h1sq, NK)
            sb1 = scale_bias_from_mv(mv1)

            if has_conv2:
                # pass 2: re-gather, normalize+lrelu, conv2, running max + sums
                g1s = small("g1s", (128, KNN))
                ssq = small("h2sq", (128, KNN))
                sscr = chpool.tile([128, HB], F32, tag="sscr", bufs=1)
                for q in range(KNN // 2):
                    gch = chpool.tile([128, 2 * HB], F32, tag="gch", bufs=2)
                    nc.gpsimd.ap_gather(
                        gch[:], ya3, wri[:, 256 * q:256 * (q + 1)],
                        channels=128, num_elems=N, d=1, num_idxs=2 * HB)
                    nc.vector.scalar_tensor_tensor(
                        gch[:].rearrange("p (j n) -> p j n", j=2),
                        gch[:].rearrange("p (j n) -> p j n", j=2),
                        1.0, yb_b2, ALU.mult, ALU.add)
                    for r in range(2):
                        j = 2 * q + r
                        gsl = gch[:, HB * r:HB * (r + 1)]
                        nc.scalar.activation(
                            gsl, gsl, AF.Prelu, bias=sb1[:, 1:2], scale=sb1[:, 0:1],
                            alpha=0.2, accum_out=g1s[:, j:j + 1])
                        psc = pp.tile([128, HB], F32, tag="pp")
                        for h in range(2):
                            for jj in range(4):
                                sl = slice(512 * jj, 512 * (jj + 1))
                                nc.tensor.matmul(
                                    psc[64 * h:64 * h + 64, sl],
                                    wcT[bi][64 * h:64 * h + 64, :],
                                    gch[64 * h:64 * h + 64, HB * r + 512 * jj:
                                        HB * r + 512 * (jj + 1)])
                        nc.vector.tensor_tensor(M[:], M[:], psc[:, 0:HB], ALU.max)
                        nc.scalar.activation(
                            sscr[:], psc[:, 0:HB], AF.Square, accum_out=ssq[:, j:j + 1])
                # sum(h2) per channel-half = W2 @ sum(g) (tiny matmuls)
                gsum = small("gsum", (128, 1))
                nc.vector.tensor_reduce(gsum[:], g1s[:], axis=AX.X, op=ALU.add)
                pss = pp.tile([128, HB], F32, tag="pp")
                nc.tensor.matmul(pss[0:64, 0:1], wcT[bi][0:64, :], gsum[0:64, 0:1])
                nc.tensor.matmul(pss[64:128, 0:1], wcT[bi][64:128, :], gsum[64:128, 0:1])
                ssum = small("h2sum", (128, 1))
                nc.scalar.copy(ssum[:], pss[:, 0:1])
                mv2 = mv_from_sums(ssum, ssq, NK)
                sb2 = scale_bias_from_mv(mv2)
                xout_p = x_p[bi + 1]
                nc.scalar.activation(
                    xout_p[:].rearrange("p (t g q) -> p g t q", t=16, g=8, q=16),
                    M[:], AF.Prelu, bias=sb2[:, 1:2], scale=sb2[:, 0:1], alpha=0.2)
            else:
                xout_p = x_p[bi + 1]
                nc.scalar.activation(
                    xout_p[:].rearrange("p (t g q) -> p g t q", t=16, g=8, q=16),
                    M[:], AF.Prelu, bias=sb1[:, 1:2], scale=sb1[:, 0:1], alpha=0.2)

            return xout_p

        xp = x_p[0]
        for bi in range(3):
            xp = edge_block(bi, xp, 4 if bi == 0 else 64)
        ectx.close()

        # ---------------- head ----------------
        x1_p, x2_p, x3_p = x_p[1], x_p[2], x_p[3]
        hb_pool = ctx.enter_context(tc.tile_pool(name="hb_pool", bufs=1))
        hu_pool = ctx.enter_context(tc.tile_pool(name="hu_pool", bufs=3))
        xb = []
        for i, xpp in enumerate([x1_p, x2_p, x3_p]):
            t = hb_pool.tile([128, HB], BF16, tag=f"xb{i}")
            nc.scalar.copy(t[:], xpp[:])
            xb.append(t)

        def head_norm_stats(u, hs, nseg):
            """hs [128, nseg]: per-segment sums collected via accum_out on the
            PSUM-evacuation copies; the squares pass runs on the (head-idle)
            DVE so the ACT engine stays free for the copies/prelus."""
            hq = small("hq", (128, 1))
            uscr = hu_pool.tile([128, N], F32, tag="uscr", bufs=1, name="uscr")
            nc.vector.tensor_tensor_reduce(
                uscr[:], u[:], u[:], 1.0, 0.0, ALU.mult, ALU.add,
                accum_out=hq[:, :])
            mv = small("mvh", (128, 2))
            nc.vector.tensor_reduce(mv[:, 0:1], hs[:, 0:nseg], axis=AX.X, op=ALU.add)
            ts(mv[:, 0:1], mv[:, 0:1], 1.0 / N, ALU.mult)
            ts(mv[:, 1:2], hq[:, :], 1.0 / N, ALU.mult)
            m2 = small("hm2")
            nc.vector.tensor_tensor(m2[:], mv[:, 0:1], mv[:, 0:1], ALU.mult)
            nc.vector.tensor_tensor(mv[:, 1:2], mv[:, 1:2], m2[:], ALU.subtract)
            ts(mv[:, 1:2], mv[:, 1:2], 1e-5, ALU.add)
            s = small("sh"); b = small("bh")
            rsqrt_inplace(s[:], mv[:, 1:2], 128)
            nc.vector.tensor_tensor(b[:], mv[:, 0:1], s[:], ALU.mult)
            ts(b[:], b[:], -1.0, ALU.mult)
            return s, b

        gvecb = hb_pool.tile([128, 8], BF16, tag="gvecb")
        for g in range(8):
            u6 = hu_pool.tile([128, N], F32, tag="uh", name="u6")
            hs6 = small("hs_seg", (128, 2))
            for h in range(2):
                ps6 = pp.tile([128, HB], F32, tag="pp")
                for ci in range(4):
                    sl = slice(512 * ci, 512 * (ci + 1))
                    for ki in range(3):
                        nc.tensor.matmul(
                            ps6[:, sl],
                            w6T[ki][64 * h:64 * h + 64, 128 * g:128 * (g + 1)],
                            xb[ki][64 * h:64 * h + 64, sl],
                            start=(ki == 0), stop=(ki == 2))
                nc.scalar.activation(
                    u6[:, HB * h:HB * (h + 1)], ps6[:, 0:HB], AF.Copy,
                    accum_out=hs6[:, h:h + 1])
            s, b = head_norm_stats(u6, hs6, 2)
            # prelu(s*x+b) is monotone in x (s>0), so reduce-max first (on the
            # otherwise-idle Pool engine) and normalize just the maxima.
            gmax = small("gmax")
            nc.vector.tensor_reduce(gmax[:], u6[:], axis=AX.X, op=ALU.max)
            gact = small("gact")
            nc.scalar.activation(
                gact[:], gmax[:], AF.Prelu, bias=b[:, :], scale=s[:, :], alpha=0.2)
            nc.scalar.copy(gvecb[:, g:g + 1], gact[:])

        bias7 = hb_pool.tile([128, 4], F32, tag="bias7")
        ps7b = pp.tile([128, HB], F32, tag="pp")
        for og in range(4):
            for g in range(8):
                nc.tensor.matmul(
                    ps7b[:, og:og + 1],
                    w7gT[g][:, 128 * og:128 * (og + 1)],
                    gvecb[:, g:g + 1],
                    start=(g == 0), stop=(g == 7))
        nc.scalar.copy(bias7[:], ps7b[:, 0:4])

        h7b = []
        for og in range(4):
            u7 = hu_pool.tile([128, N], F32, tag="uh", name="u7")
            hs7 = small("hs_seg", (128, 2))
            for h in range(2):
                ps7 = pp.tile([128, HB], F32, tag="pp")
                for ci in range(4):
                    sl = slice(512 * ci, 512 * (ci + 1))
                    for ki in range(3):
                        nc.tensor.matmul(
                            ps7[:, sl],
                            w7xT[ki][64 * h:64 * h + 64, 128 * og:128 * (og + 1)],
                            xb[ki][64 * h:64 * h + 64, sl],
                            start=(ki == 0), stop=(ki == 2))
                nc.scalar.activation(
                    u7[:, HB * h:HB * (h + 1)], ps7[:, 0:HB],
                    AF.Identity, bias=bias7[:, og:og + 1],
                    accum_out=hs7[:, h:h + 1])
            s, b = head_norm_stats(u7, hs7, 2)
            t = hb_pool.tile([128, N], BF16, tag=f"h7b{og}")
            nc.scalar.activation(t[:], u7[:], AF.Prelu, bias=b[:, :], scale=s[:, :], alpha=0.2)
            h7b.append(t)

        h8b = []
        for og in range(2):
            u8 = hu_pool.tile([128, N], F32, tag="uh", name="u8")
            hs8 = small("hs_seg", (128, 2))
            for h in range(2):
                ps8 = pp.tile([128, HB], F32, tag="pp")
                for ci in range(4):
                    sl = slice(HB * h + 512 * ci, HB * h + 512 * (ci + 1))
                    for ki in range(4):
                        nc.tensor.matmul(
                            ps8[:, 512 * ci:512 * (ci + 1)],
                            w8T[ki][:, 128 * og:128 * (og + 1)],
                            h7b[ki][:, sl],
                            start=(ki == 0), stop=(ki == 3))
                nc.scalar.activation(
                    u8[:, HB * h:HB * (h + 1)], ps8[:, 0:HB], AF.Copy,
                    accum_out=hs8[:, h:h + 1])
            s, b = head_norm_stats(u8, hs8, 2)
            t = hb_pool.tile([128, N], BF16, tag=f"h8b{og}")
            nc.scalar.activation(t[:], u8[:], AF.Prelu, bias=b[:, :], scale=s[:, :], alpha=0.2)
            h8b.append(t)

        o2 = hu_pool.tile([2, N], F32, tag="uh", name="o2")
        for ci in range(8):
            sl = slice(512 * ci, 512 * (ci + 1))
            ps9 = pp.tile([128, HB], F32, tag="pp")
            for ki in range(2):
                nc.tensor.matmul(
                    ps9[0:2, 0:512],
                    w9T[ki][:], h8b[ki][:, sl],
                    start=(ki == 0), stop=(ki == 1))
            nc.scalar.copy(o2[:, sl], ps9[0:2, 0:512])

        ost = hb_pool.tile([128, 64], F32, tag="ost")
        pst = pp.tile([128, HB], F32, tag="pp")
        for t in range(NT):
            nc.tensor.transpose(
                pst[:, 2 * t:2 * (t + 1)], o2[:, 128 * t:128 * (t + 1)], id2[:])
        nc.scalar.copy(ost[:], pst[:, 0:64])
        nc.sync.dma_start(
            out_d.rearrange("(t p) c -> p t c", p=128),
            ost[:].rearrange("p (t c) -> p t c", c=2))

    nc.finalize()
    return nc


def _shared_inputs(ws):
    import ml_dtypes
    w1, w2, w3, w4, w5, w6, w7, w8, w9 = ws
    f32 = np.float32
    bf16 = ml_dtypes.bfloat16
    d = {}
    for i, w in [(1, w1), (3, w3), (5, w5)]:
        C = w.shape[1] // 2
        d[f"waT{i}"] = np.ascontiguousarray(w[:, :C].T.astype(f32))
        d[f"wbT{i}"] = np.ascontiguousarray((w[:, C:] - w[:, :C]).T.astype(f32))
    d["w2T"] = np.ascontiguousarray(w2.T.astype(f32))
    d["w4T"] = np.ascontiguousarray(w4.T.astype(f32))
    w6t = w6.T.astype(bf16); w7gt = w7[:, :1024].T.astype(bf16)
    w7xt = w7[:, 1024:].T.astype(bf16); w8t = w8.T.astype(bf16)
    w9t = w9.T.astype(bf16)
    for k in range(3):
        d[f"w6T_{k}"] = np.ascontiguousarray(w6t[64 * k:64 * (k + 1)])
        d[f"w7xT_{k}"] = np.ascontiguousarray(w7xt[64 * k:64 * (k + 1)])
    for k in range(8):
        d[f"w7gT_{k}"] = np.ascontiguousarray(w7gt[128 * k:128 * (k + 1)])
    for k in range(4):
        d[f"w8T_{k}"] = np.ascontiguousarray(w8t[128 * k:128 * (k + 1)])
    for k in range(2):
        d[f"w9T_{k}"] = np.ascontiguousarray(w9t[128 * k:128 * (k + 1)])
    d["iota256"] = np.broadcast_to(
        np.arange(CHUNK, dtype=np.uint32)[None, :], (128, CHUNK)).copy()
    d["maskc"] = np.full((128, 1), 0xFFFFFF00, dtype=np.uint32)
    d["magic"] = np.full((128, 1), 0x5F3759DF, dtype=np.uint32)
    d["id2"] = np.eye(2, dtype=f32)
    return d


def _run(inputs, want_debug=False):
    from concourse.bass_utils import run_bass_kernel_spmd

    if "nc" not in _CACHE:
        _CACHE["nc"] = _build_program()
    nc = _CACHE["nc"]

    x = np.asarray(inputs["x"], dtype=np.float32)
    ws = [np.asarray(inputs[f"w{i}"], dtype=np.float32) for i in range(1, 10)]
    shared = _shared_inputs(ws)
    in_maps = []
    for c in range(8):
        m = dict(shared)
        m["xt"] = np.ascontiguousarray(x[c % 4].T.astype(np.float32))
        in_maps.append(m)
    res = run_bass_kernel_spmd(nc, in_maps, list(range(8)))
    out = np.stack([res.results[c]["out"] for c in range(4)])
    if want_debug:
        return out, [res.results[c] for c in range(4)]
    return out


def kernel(**inputs):
    return _run(inputs)



# revision 22
# speedup vs baseline: 1.3351x; 1.3351x over previous
"""DGCNN semantic-segmentation kernel for 8x Trainium2 NeuronCores.

Strategy: each sample is split across a PAIR of cores (8 cores / 4 samples).
Core 2s+h computes queries [2048h, 2048h+2048) of sample s: the kNN
selection, gathers and convs run over the core's 2048 queries against the
full 4096-point candidate set, so per-query math is bit-identical to a
single-core run at half the per-core cost. At each block boundary the pair
exchanges its packed x-half via a paired DRAM AllGather; instance-norm
stats are combined across the pair the same way. The global head (w6..w9)
is cheap and duplicated on both cores of a pair from the exchanged full
tensors; the host reads each sample's output from its even core.

Per-sample pipeline: 3x EdgeConv blocks: kNN (fp16 hi/lo split K=13
distance matmul, ~1e-6 accurate) -> top-20 selection (pack 8-bit
chunk-local index into low mantissa bits, top-8 per 256-chunk via DVE max8,
refine via max8/match_replace, recover columns arithmetically) -> streamed
gather via GPSIMD ap_gather (4 k per instr) -> folded 1x1 conv +
instance-norm + leaky-relu -> second conv (fp32 matmul) streamed with
running max over k (max commutes with the monotone normalize+lrelu).

Local layout: packed [128, 1024] fp32 - partition p<64 holds channel p for
the core's queries 0..1023, partition 64+p for queries 1024..2047. The
exchanged full tensors use the global packed [128, 2048] layout.
"""

import sys

if "/opt/trn_rl_repo" not in sys.path:
    sys.path.insert(0, "/opt/trn_rl_repo")

import numpy as np

N = 4096         # candidate points (full sample)
HB = 2048        # global packed free size (full tensors)
LHB = 1024       # local packed free size (2048 queries / 2 partition halves)
LNT = 16         # row tiles of 128 local queries
KNN = 20
NKL = KNN * LHB  # per-partition element count behind each stat sum
CHUNK = 256
NEG = -3.0e38
PAIRS = [[0, 1], [2, 3], [4, 5], [6, 7]]

_CACHE = {}


def _build_program():
    import concourse.bacc as bacc
    import concourse.tile as tile
    from concourse import mybir
    from contextlib import ExitStack

    F32 = mybir.dt.float32
    F16 = mybir.dt.float16
    BF16 = mybir.dt.bfloat16
    U32 = mybir.dt.uint32
    U16 = mybir.dt.uint16
    I16 = mybir.dt.int16
    AF = mybir.ActivationFunctionType
    ALU = mybir.AluOpType
    AX = mybir.AxisListType

    nc = bacc.Bacc("TRN2", target_bir_lowering=False, debug=False, num_devices=8)

    def din(name, shape, dt=F32):
        return nc.dram_tensor(name, shape, dt, kind="ExternalInput").ap()

    xt_d = din("xt", [4, HB])
    xtf_d = din("xtf", [4, N])
    waT_d = [din("waT1", [4, 64]), din("waT3", [64, 64]), din("waT5", [64, 64])]
    wbT_d = [din("wbT1", [4, 64]), din("wbT3", [64, 64]), din("wbT5", [64, 64])]
    wcT_d = [din("w2T", [64, 64]), din("w4T", [64, 64])]
    w6T_d = [din(f"w6T_{k}", [64, 1024], BF16) for k in range(3)]
    w7gT_d = [din(f"w7gT_{k}", [128, 512], BF16) for k in range(8)]
    w7xT_d = [din(f"w7xT_{k}", [64, 512], BF16) for k in range(3)]
    w8T_d = [din(f"w8T_{k}", [128, 256], BF16) for k in range(4)]
    w9T_d = [din(f"w9T_{k}", [128, 2], BF16) for k in range(2)]
    iota256_d = din("iota256", [128, CHUNK], U32)
    maskc_d = din("maskc", [128, 1], U32)
    id2_d = din("id2", [2, 2])

    out_d = nc.dram_tensor("out", [N, 2], F32, kind="ExternalOutput").ap()

    # DRAM staging for the pairwise exchanges (AllGather concatenates the two
    # cores' buffers rank-major along rows).
    xsh_in = [nc.dram_tensor(f"xsh_in{b}", [128, LHB], F32).ap() for b in range(3)]
    xsh_out = [nc.dram_tensor(f"xsh_out{b}", [256, LHB], F32).ap() for b in range(3)]
    msh_in = [nc.dram_tensor(f"msh_in{k}", [128, 2], F32).ap() for k in range(5)]
    msh_out = [nc.dram_tensor(f"msh_out{k}", [256, 2], F32).ap() for k in range(5)]
    _mvc = [0]

    with tile.TileContext(nc) as tc, ExitStack() as ctx:
        wpool = ctx.enter_context(tc.tile_pool(name="wpool", bufs=1))
        xpool = ctx.enter_context(tc.tile_pool(name="xpool", bufs=1))
        stpool = ctx.enter_context(tc.tile_pool(name="stpool", bufs=1))
        pp = ctx.enter_context(tc.tile_pool(name="pp", bufs=2, space="PSUM"))
        xfpool = ctx.enter_context(tc.tile_pool(name="xfpool", bufs=1))
        ectx = ExitStack()
        abpool = ectx.enter_context(tc.tile_pool(name="abpool", bufs=1))
        gpool = ectx.enter_context(tc.tile_pool(name="gpool", bufs=1))
        dpool = ectx.enter_context(tc.tile_pool(name="dpool", bufs=3))
        selpool = ectx.enter_context(tc.tile_pool(name="selpool", bufs=4))
        idxpool = ectx.enter_context(tc.tile_pool(name="idxpool", bufs=1))
        chpool = ectx.enter_context(tc.tile_pool(name="chpool", bufs=1))

        def load(pool, ap_d, shape, dt=F32, dup64=False):
            rows = shape[0]
            tshape = [128, shape[1]] if dup64 else shape
            t = pool.tile(tshape, dt, tag=ap_d.tensor.name, name=ap_d.tensor.name + "_sb")
            nc.sync.dma_start(t[0:rows, :], ap_d)
            if dup64:
                nc.sync.dma_start(t[64:64 + rows, :], ap_d)
            return t

        waT = [load(wpool, waT_d[0], [4, 64], dup64=True),
               load(wpool, waT_d[1], [64, 64], dup64=True),
               load(wpool, waT_d[2], [64, 64], dup64=True)]
        wbT = [load(wpool, wbT_d[0], [4, 64], dup64=True),
               load(wpool, wbT_d[1], [64, 64], dup64=True),
               load(wpool, wbT_d[2], [64, 64], dup64=True)]
        wcT = [load(wpool, wcT_d[0], [64, 64], dup64=True),
               load(wpool, wcT_d[1], [64, 64], dup64=True), None]
        iota256 = load(wpool, iota256_d, [128, CHUNK], U32)
        maskc = load(wpool, maskc_d, [128, 1], U32)
        id2 = load(wpool, id2_d, [2, 2])
        w6T = [load(wpool, a, [64, 1024], BF16, dup64=True) for a in w6T_d]
        w7gT = [load(wpool, a, [128, 512], BF16) for a in w7gT_d]
        w7xT = [load(wpool, a, [64, 512], BF16, dup64=True) for a in w7xT_d]
        w8T = [load(wpool, a, [128, 256], BF16) for a in w8T_d]
        w9T = [load(wpool, a, [128, 2], BF16) for a in w9T_d]

        # block-1 inputs: local half + full sample (both host-fed)
        xt_sb = gpool.tile([4, HB], F32, tag="ya_dup", name="xt_sb")
        nc.sync.dma_start(xt_sb[:], xt_d)
        xtf_sb = gpool.tile([4, N], F32, tag="ya_dup", name="xtf_sb")
        nc.sync.dma_start(xtf_sb[:], xtf_d)
        xl_p = [xpool.tile([128, LHB], F32, tag=f"xl{i}_p", name=f"xl{i}_p")
                for i in range(4)]
        nc.sync.dma_start(xl_p[0][0:4, :], xt_sb[:, 0:LHB])
        nc.sync.dma_start(xl_p[0][64:68, :], xt_sb[:, LHB:HB])
        xf_p = [xfpool.tile([128, HB], F32, tag=f"xf{i}_p", name=f"xf{i}_p")
                for i in range(4)]
        nc.sync.dma_start(xf_p[0][0:4, :], xtf_sb[:, 0:HB])
        nc.sync.dma_start(xf_p[0][64:68, :], xtf_sb[:, HB:N])

        def small(tag, shape=(128, 1), dt=F32):
            return stpool.tile(list(shape), dt, tag=tag, name=tag)

        def ts(out, in0, s1, op0, s2=None, op1=None):
            if op1 is None:
                nc.vector.tensor_scalar(out, in0, s1, None, op0)
            else:
                nc.vector.tensor_scalar(out, in0, s1, s2, op0, op1)

        def rsqrt_inplace(y, t_in, rows):
            r = small("rs_r", (rows, 1))
            nc.vector.reciprocal(r[:], t_in)
            nc.scalar.activation(y, r[:], AF.Sqrt)

        def mv_from_sums(ssum, ssq, count):
            """[128, w] partial sums -> mv [128, 2] = (mean, E[x^2]) per
            partition (each covering `count` elements)."""
            mv = small("sm_mv", (128, 2))
            nc.vector.tensor_reduce(mv[:, 0:1], ssum[:], axis=AX.X, op=ALU.add)
            nc.vector.tensor_reduce(mv[:, 1:2], ssq[:], axis=AX.X, op=ALU.add)
            ts(mv[:], mv[:], 1.0 / count, ALU.mult)
            return mv

        def scale_bias_from_mv(mv):
            """mv [128,2] local (mean, E2); pair-exchange via AllGather, then
            combine the 4 equal-count groups (2 cores x 2 partition halves)
            into per-channel scale/bias for normalized = s*x + b."""
            k = _mvc[0]
            _mvc[0] += 1
            nc.sync.dma_start(msh_in[k], mv[:])
            nc.gpsimd.collective_compute(
                "AllGather", ALU.bypass, replica_groups=PAIRS,
                ins=[msh_in[k]], outs=[msh_out[k]])
            mvp = small("st_mvp", (128, 4))
            nc.sync.dma_start(mvp[:, 0:2], msh_out[k][0:128, :])
            nc.sync.dma_start(mvp[:, 2:4], msh_out[k][128:256, :])
            mvs = small("st_mvs", (128, 2))
            nc.vector.tensor_tensor(mvs[:], mvp[:, 0:2], mvp[:, 2:4], ALU.add)
            mvb = small("st_mvb", (64, 2))
            nc.sync.dma_start(mvb[:], mvs[64:128, :])
            me = small("st_me", (64, 2))
            nc.vector.tensor_tensor(me[:], mvs[0:64, :], mvb[:], ALU.add)
            ts(me[:], me[:], 0.25, ALU.mult)
            v = small("st_v", (64, 1))
            nc.vector.tensor_tensor(v[:], me[:, 0:1], me[:, 0:1], ALU.mult)
            nc.vector.tensor_tensor(v[:], me[:, 1:2], v[:], ALU.subtract)
            ts(v[:], v[:], 1e-5, ALU.add)
            s = small("st_s", (64, 1))
            rsqrt_inplace(s[:], v[:], 64)
            bb = small("st_bb", (64, 1))
            nc.vector.tensor_tensor(bb[:], me[:, 0:1], s[:], ALU.mult)
            ts(bb[:], bb[:], -1.0, ALU.mult)
            sb128 = small("st_sb128", (128, 2))
            nc.vector.tensor_copy(sb128[0:64, 0:1], s[:])
            nc.vector.tensor_copy(sb128[0:64, 1:2], bb[:])
            nc.sync.dma_start(sb128[64:128, :], sb128[0:64, :])
            return sb128

        # ---------------- EdgeConv block ----------------
        def edge_block(bi, xl, xf, C):
            has_conv2 = bi < 2

            ones3 = abpool.tile([128, 1], F32, tag="ones3")
            nc.gpsimd.memset(ones3[:], 1.0)

            # -- candidate-side (full-N) operand prep from xf --
            sq = dpool.tile([128, HB], F32, tag="dpk", name="sq")
            nc.scalar.activation(sq[0:3, :], xf[0:3, :], AF.Square)
            nc.scalar.activation(sq[64:67, :], xf[64:67, :], AF.Square)
            src4 = gpool.tile([4, N], F32, tag="ya_dup", name="src4")
            for h in range(2):
                psx = pp.tile([128, HB], F32, tag="pp")
                for j in range(4):
                    nc.tensor.matmul(
                        psx[0:1, 512 * j:512 * (j + 1)],
                        ones3[64 * h:64 * h + 3, 0:1],
                        sq[64 * h:64 * h + 3, 512 * j:512 * (j + 1)])
                nc.scalar.copy(src4[0:1, HB * h:HB * (h + 1)], psx[0:1, 0:HB])
                nc.sync.dma_start(src4[1:4, HB * h:HB * (h + 1)], xf[64 * h:64 * h + 3, :])

            hi4 = gpool.tile([4, N], F16, tag="M", name="hi4")
            lo4 = gpool.tile([4, N], F16, tag="yb_p", name="lo4")
            nc.scalar.copy(hi4[:], src4[:])
            nc.vector.scalar_tensor_tensor(
                lo4[:], hi4[:], -1.0, src4[:], ALU.mult, ALU.add)

            # bT rows: [-xxh, -xxl, 1, 1, ph(3), pl(3), ph(3)]
            bT = abpool.tile([16, N], F16, tag="bT")
            nh4 = dpool.tile([4, N], F16, tag="dpk", name="nh4")
            nc.scalar.mul(nh4[:], hi4[:], -1.0)
            nl4 = dpool.tile([4, N], F16, tag="dpk", name="nl4")
            nc.scalar.mul(nl4[:], lo4[:], -1.0)
            nc.gpsimd.memset(bT[2:4, :], 1.0)
            nc.sync.dma_start(bT[0:1, :], nh4[0:1, :])
            nc.sync.dma_start(bT[1:2, :], nl4[0:1, :])
            nc.sync.dma_start(bT[4:7, :], hi4[1:4, :])
            nc.sync.dma_start(bT[7:10, :], lo4[1:4, :])
            nc.sync.dma_start(bT[10:13, :], hi4[1:4, :])

            # -- query-side (local) operand prep from xl --
            sql = dpool.tile([128, LHB], F32, tag="sql", bufs=1, name="sql")
            nc.scalar.activation(sql[0:3, :], xl[0:3, :], AF.Square)
            nc.scalar.activation(sql[64:67, :], xl[64:67, :], AF.Square)
            src4l = gpool.tile([4, HB], F32, tag="yb_p", name="src4l")
            for h in range(2):
                psx = pp.tile([128, HB], F32, tag="pp")
                for j in range(2):
                    nc.tensor.matmul(
                        psx[0:1, 512 * j:512 * (j + 1)],
                        ones3[64 * h:64 * h + 3, 0:1],
                        sql[64 * h:64 * h + 3, 512 * j:512 * (j + 1)])
                nc.scalar.copy(src4l[0:1, LHB * h:LHB * (h + 1)], psx[0:1, 0:LHB])
                nc.sync.dma_start(src4l[1:4, LHB * h:LHB * (h + 1)], xl[64 * h:64 * h + 3, :])

            hi4l = gpool.tile([4, HB], F16, tag="hi4l", name="hi4l")
            lo4l = gpool.tile([4, HB], F16, tag="lo4l", name="lo4l")
            nc.scalar.copy(hi4l[:], src4l[:])
            nc.vector.scalar_tensor_tensor(
                lo4l[:], hi4l[:], -1.0, src4l[:], ALU.mult, ALU.add)

            # aT rows: [1, 1, -xxh, -xxl, 2ph(3), 2ph(3), 2pl(3)]
            aT = abpool.tile([16, HB], F16, tag="aT")
            nh4l = dpool.tile([4, HB], F16, tag="dpk", name="nh4l")
            nl4l = dpool.tile([4, HB], F16, tag="dpk", name="nl4l")
            nc.scalar.mul(nh4l[:], hi4l[:], -1.0)
            nc.scalar.mul(nl4l[:], lo4l[:], -1.0)
            nc.gpsimd.memset(aT[0:2, :], 1.0)
            nc.sync.dma_start(aT[2:3, :], nh4l[0:1, :])
            nc.sync.dma_start(aT[3:4, :], nl4l[0:1, :])
            h2x = dpool.tile([4, HB], F16, tag="dpk", name="h2x")
            l2x = dpool.tile([4, HB], F16, tag="dpk", name="l2x")
            nc.scalar.mul(h2x[:], hi4l[:], 2.0)
            nc.scalar.mul(l2x[:], lo4l[:], 2.0)
            nc.sync.dma_start(aT[4:7, :], h2x[1:4, :])
            nc.sync.dma_start(aT[7:10, :], h2x[1:4, :])
            nc.sync.dma_start(aT[10:13, :], l2x[1:4, :])

            # ya (full n, duplicated to both partition halves, from xf) and
            # yb (local, packed in gather order)
            ya_dup = gpool.tile([128, N], F32, tag="ya_dup")
            yb_p = gpool.tile([128, LHB], F32, tag="yb_p")
            for dsth in range(2):
                po = 64 * dsth
                psy = pp.tile([128, HB], F32, tag="pp")
                for srch in range(2):
                    for j in range(4):
                        sl = slice(512 * j, 512 * (j + 1))
                        nc.tensor.matmul(
                            psy[po:po + 64, sl],
                            waT[bi][64 * srch:64 * srch + C, :],
                            xf[64 * srch:64 * srch + C, sl])
                    nc.scalar.copy(
                        ya_dup[po:po + 64, HB * srch:HB * (srch + 1)],
                        psy[po:po + 64, 0:HB])
                psb = pp.tile([128, HB], F32, tag="pp")
                for j in range(2):
                    sl = slice(512 * j, 512 * (j + 1))
                    nc.tensor.matmul(
                        psb[po:po + 64, sl],
                        wbT[bi][64 * dsth:64 * dsth + C, :],
                        xl[64 * dsth:64 * dsth + C, sl])
                nc.scalar.copy(
                    yb_p[po:po + 64, :].rearrange(
                        "p (g t q) -> p t g q", g=8, t=8, q=16),
                    psb[po:po + 64, 0:LHB].rearrange(
                        "p (t g q) -> p t g q", t=8, g=8, q=16))

            # ---- distance + selection over 16 local query tiles ----
            colbuf = idxpool.tile([128, 384], U16, tag="colbuf")
            posall = idxpool.tile([128, 384], U16, tag="posall")
            v24all = idxpool.tile([128, 384], F32, tag="v24all")
            for t in range(LNT):
                lhs = aT[0:13, 128 * t:128 * (t + 1)]
                cand = selpool.tile([128, 128], F32, tag="cand")
                for h in range(2):
                    psd = pp.tile([128, HB], F32, tag="pp")
                    for j in range(4):
                        nc.tensor.matmul(
                            psd[:, 512 * j:512 * (j + 1)], lhs,
                            bT[0:13, HB * h + 512 * j:HB * h + 512 * (j + 1)])
                    dpk = dpool.tile([128, HB], U32, tag="dpk")
                    nc.vector.scalar_tensor_tensor(
                        dpk[:].rearrange("p (a c) -> p a c", c=CHUNK),
                        psd[:].bitcast(U32).rearrange("p (a c) -> p a c", c=CHUNK),
                        maskc[:, :],
                        iota256[:].rearrange("p (a c) -> p a c", a=1).broadcast_to([128, 8, CHUNK]),
                        ALU.bitwise_and, ALU.bitwise_or)
                    for c in range(8):
                        nc.vector.max(
                            cand[:, 64 * h + 8 * c:64 * h + 8 * (c + 1)],
                            dpk[:].bitcast(F32)[:, CHUNK * c:CHUNK * (c + 1)])
                v24 = v24all[:, 24 * t:24 * (t + 1)]
                pos = posall[:, 24 * t:24 * (t + 1)]
                c2 = selpool.tile([128, 128], F32, tag="c2")
                c3 = selpool.tile([128, 128], F32, tag="c3")
                nc.vector.max(v24[:, 0:8], cand[:])
                nc.vector.match_replace(c2[:], v24[:, 0:8], cand[:], NEG)
                nc.vector.max(v24[:, 8:16], c2[:])
                nc.vector.match_replace(c3[:], v24[:, 8:16], c2[:], NEG)
                nc.vector.max(v24[:, 16:24], c3[:])
                nc.vector.max_index(pos[:, 0:8], v24[:, 0:8], cand[:])
                nc.vector.max_index(pos[:, 8:16], v24[:, 8:16], cand[:])
                nc.vector.max_index(pos[:, 16:24], v24[:, 16:24], cand[:])
            # batched column arithmetic: col = (pos>>3)*256 + (v24.bits & 255)
            locb = idxpool.tile([128, 384], U32, tag="locb")
            ts(locb[:], v24all[:].bitcast(U32), 255, ALU.bitwise_and)
            loc16b = idxpool.tile([128, 384], U16, tag="loc16b")
            nc.vector.tensor_copy(loc16b[:], locb[:])
            ts(posall[:], posall[:], 3, ALU.logical_shift_right)
            ts(posall[:], posall[:], 8, ALU.logical_shift_left)
            nc.vector.tensor_tensor(
                colbuf[:].rearrange("p (j t) -> p t j", j=24),
                posall[:].rearrange("p (t j) -> p t j", j=24),
                loc16b[:].rearrange("p (t j) -> p t j", j=24), ALU.add)

            # ---- reformat into per-core wrapped gather index lists ----
            # per local subhalf hl: position i = j*1024 + n_loc; stored
            # wrapped-16: partition 16k + (i%16), free i//16 = j*64 + g2*8 + t
            wrapped = idxpool.tile([128, KNN * 64], U16, tag="wrapped")
            for hl in range(2):
                for g2 in range(8):
                    src = colbuf[16 * g2:16 * (g2 + 1), :] \
                        .rearrange("p (j t) -> p j t", t=LNT)[:, 0:KNN, 8 * hl:8 * (hl + 1)]
                    dst = wrapped[64 * hl:64 * hl + 16, :] \
                        .rearrange("p (j g t) -> p j g t", g=8, t=8)[:, :, g2, :]
                    nc.sync.dma_start(dst, src)
                for k in range(1, 4):
                    nc.sync.dma_start(
                        wrapped[64 * hl + 16 * k:64 * hl + 16 * (k + 1), :],
                        wrapped[64 * hl:64 * hl + 16, :])

            ya3 = ya_dup[:].rearrange("p (m d) -> p m d", d=1)
            wri = wrapped[:].bitcast(I16)

            # ---- pass 1: streamed gather (4 k per instr) -> bn stats ----
            M = gpool.tile([128, LHB], F32, tag="M")
            nc.gpsimd.memset(M[:], NEG)
            h1sum = small("h1sum", (128, KNN // 4))
            h1sq = small("h1sq", (128, KNN))
            sscr1 = chpool.tile([128, LHB], F32, tag="sscr", bufs=1, name="sscr1")
            yb_b4 = yb_p[:].rearrange("p (o n) -> p o n", o=1).broadcast_to([128, 4, LHB])
            for q in range(KNN // 4):
                gch = chpool.tile([128, 4 * LHB], F32, tag="gch", bufs=2)
                nc.gpsimd.ap_gather(
                    gch[:], ya3, wri[:, 256 * q:256 * (q + 1)],
                    channels=128, num_elems=N, d=1, num_idxs=4 * LHB)
                nc.vector.scalar_tensor_tensor(
                    gch[:].rearrange("p (j n) -> p j n", j=4),
                    gch[:].rearrange("p (j n) -> p j n", j=4),
                    1.0, yb_b4, ALU.mult, ALU.add,
                    accum_out=h1sum[:, q:q + 1])
                for r in range(4):
                    nc.scalar.activation(
                        sscr1[:], gch[:, LHB * r:LHB * (r + 1)], AF.Square,
                        accum_out=h1sq[:, 4 * q + r:4 * q + r + 1])
                    if not has_conv2:
                        nc.vector.tensor_tensor(
                            M[:], M[:], gch[:, LHB * r:LHB * (r + 1)], ALU.max)
            mv1 = mv_from_sums(h1sum, h1sq, NKL)
            sb1 = scale_bias_from_mv(mv1)

            if has_conv2:
                # pass 2: re-gather, normalize+lrelu, conv2, running max + sums
                g1s = small("g1s", (128, KNN))
                ssq = small("h2sq", (128, KNN))
                sscr = chpool.tile([128, LHB], F32, tag="sscr", bufs=1)
                for q in range(KNN // 4):
                    gch = chpool.tile([128, 4 * LHB], F32, tag="gch", bufs=2)
                    nc.gpsimd.ap_gather(
                        gch[:], ya3, wri[:, 256 * q:256 * (q + 1)],
                        channels=128, num_elems=N, d=1, num_idxs=4 * LHB)
                    nc.vector.scalar_tensor_tensor(
                        gch[:].rearrange("p (j n) -> p j n", j=4),
                        gch[:].rearrange("p (j n) -> p j n", j=4),
                        1.0, yb_b4, ALU.mult, ALU.add)
                    for r in range(4):
                        j = 4 * q + r
                        gsl = gch[:, LHB * r:LHB * (r + 1)]
                        nc.scalar.activation(
                            gsl, gsl, AF.Prelu, bias=sb1[:, 1:2], scale=sb1[:, 0:1],
                            alpha=0.2, accum_out=g1s[:, j:j + 1])
                        psc = pp.tile([128, HB], F32, tag="pp")
                        for h in range(2):
                            for jj in range(2):
                                sl = slice(512 * jj, 512 * (jj + 1))
                                nc.tensor.matmul(
                                    psc[64 * h:64 * h + 64, sl],
                                    wcT[bi][64 * h:64 * h + 64, :],
                                    gch[64 * h:64 * h + 64, LHB * r + 512 * jj:
                                        LHB * r + 512 * (jj + 1)])
                        nc.vector.tensor_tensor(M[:], M[:], psc[:, 0:LHB], ALU.max)
                        nc.scalar.activation(
                            sscr[:], psc[:, 0:LHB], AF.Square, accum_out=ssq[:, j:j + 1])
                # sum(h2) per channel-half = W2 @ sum(g) (tiny matmuls)
                gsum = small("gsum", (128, 1))
                nc.vector.tensor_reduce(gsum[:], g1s[:], axis=AX.X, op=ALU.add)
                pss = pp.tile([128, HB], F32, tag="pp")
                nc.tensor.matmul(pss[0:64, 0:1], wcT[bi][0:64, :], gsum[0:64, 0:1])
                nc.tensor.matmul(pss[64:128, 0:1], wcT[bi][64:128, :], gsum[64:128, 0:1])
                ssum = small("h2sum", (128, 1))
                nc.scalar.copy(ssum[:], pss[:, 0:1])
                mv2 = mv_from_sums(ssum, ssq, NKL)
                sb2 = scale_bias_from_mv(mv2)
                sbf = sb2
            else:
                sbf = sb1
            xout = xl_p[bi + 1]
            nc.scalar.activation(
                xout[:].rearrange("p (t g q) -> p g t q", t=8, g=8, q=16),
                M[:], AF.Prelu, bias=sbf[:, 1:2], scale=sbf[:, 0:1], alpha=0.2)

            # ---- pairwise exchange: local halves -> full packed tensor ----
            nc.sync.dma_start(xsh_in[bi], xout[:])
            nc.gpsimd.collective_compute(
                "AllGather", ALU.bypass, replica_groups=PAIRS,
                ins=[xsh_in[bi]], outs=[xsh_out[bi]])
            xfn = xf_p[bi + 1]
            nc.sync.dma_start(xfn[0:64, 0:LHB], xsh_out[bi][0:64, :])
            nc.sync.dma_start(xfn[0:64, LHB:HB], xsh_out[bi][64:128, :])
            nc.sync.dma_start(xfn[64:128, 0:LHB], xsh_out[bi][128:192, :])
            nc.sync.dma_start(xfn[64:128, LHB:HB], xsh_out[bi][192:256, :])
            return xout, xfn

        xl, xf = xl_p[0], xf_p[0]
        for bi in range(3):
            xl, xf = edge_block(bi, xl, xf, 4 if bi == 0 else 64)
        ectx.close()

        # ------------- head (duplicated on both cores of a pair) -------------
        x1_p, x2_p, x3_p = xf_p[1], xf_p[2], xf_p[3]
        hb_pool = ctx.enter_context(tc.tile_pool(name="hb_pool", bufs=1))
        hu_pool = ctx.enter_context(tc.tile_pool(name="hu_pool", bufs=3))
        xb = []
        for i, xpp in enumerate([x1_p, x2_p, x3_p]):
            t = hb_pool.tile([128, HB], BF16, tag=f"xb{i}")
            nc.scalar.copy(t[:], xpp[:])
            xb.append(t)

        def head_norm_stats(u, hs, nseg):
            """hs [128, nseg]: per-segment sums collected via accum_out on the
            PSUM-evacuation copies; the squares pass runs on the (head-idle)
            DVE so the ACT engine stays free for the copies/prelus."""
            hq = small("hq", (128, 1))
            uscr = hu_pool.tile([128, N], F32, tag="uscr", bufs=1, name="uscr")
            nc.vector.tensor_tensor_reduce(
                uscr[:], u[:], u[:], 1.0, 0.0, ALU.mult, ALU.add,
                accum_out=hq[:, :])
            mv = small("mvh", (128, 2))
            nc.vector.tensor_reduce(mv[:, 0:1], hs[:, 0:nseg], axis=AX.X, op=ALU.add)
            ts(mv[:, 0:1], mv[:, 0:1], 1.0 / N, ALU.mult)
            ts(mv[:, 1:2], hq[:, :], 1.0 / N, ALU.mult)
            m2 = small("hm2")
            nc.vector.tensor_tensor(m2[:], mv[:, 0:1], mv[:, 0:1], ALU.mult)
            nc.vector.tensor_tensor(mv[:, 1:2], mv[:, 1:2], m2[:], ALU.subtract)
            ts(mv[:, 1:2], mv[:, 1:2], 1e-5, ALU.add)
            s = small("sh"); b = small("bh")
            rsqrt_inplace(s[:], mv[:, 1:2], 128)
            nc.vector.tensor_tensor(b[:], mv[:, 0:1], s[:], ALU.mult)
            ts(b[:], b[:], -1.0, ALU.mult)
            return s, b

        gvecb = hb_pool.tile([128, 8], BF16, tag="gvecb")
        for g in range(8):
            u6 = hu_pool.tile([128, N], F32, tag="uh", name="u6")
            hs6 = small("hs_seg", (128, 2))
            for h in range(2):
                ps6 = pp.tile([128, HB], F32, tag="pp")
                for ci in range(4):
                    sl = slice(512 * ci, 512 * (ci + 1))
                    for ki in range(3):
                        nc.tensor.matmul(
                            ps6[:, sl],
                            w6T[ki][64 * h:64 * h + 64, 128 * g:128 * (g + 1)],
                            xb[ki][64 * h:64 * h + 64, sl],
                            start=(ki == 0), stop=(ki == 2))
                nc.scalar.activation(
                    u6[:, HB * h:HB * (h + 1)], ps6[:, 0:HB], AF.Copy,
                    accum_out=hs6[:, h:h + 1])
            s, b = head_norm_stats(u6, hs6, 2)
            # prelu(s*x+b) is monotone in x (s>0), so reduce-max first and
            # normalize just the maxima.
            gmax = small("gmax")
            nc.vector.tensor_reduce(gmax[:], u6[:], axis=AX.X, op=ALU.max)
            gact = small("gact")
            nc.scalar.activation(
                gact[:], gmax[:], AF.Prelu, bias=b[:, :], scale=s[:, :], alpha=0.2)
            nc.scalar.copy(gvecb[:, g:g + 1], gact[:])

        bias7 = hb_pool.tile([128, 4], F32, tag="bias7")
        ps7b = pp.tile([128, HB], F32, tag="pp")
        for og in range(4):
            for g in range(8):
                nc.tensor.matmul(
                    ps7b[:, og:og + 1],
                    w7gT[g][:, 128 * og:128 * (og + 1)],
                    gvecb[:, g:g + 1],
                    start=(g == 0), stop=(g == 7))
        nc.scalar.copy(bias7[:], ps7b[:, 0:4])

        h7b = []
        for og in range(4):
            u7 = hu_pool.tile([128, N], F32, tag="uh", name="u7")
            hs7 = small("hs_seg", (128, 2))
            for h in range(2):
                ps7 = pp.tile([128, HB], F32, tag="pp")
                for ci in range(4):
                    sl = slice(512 * ci, 512 * (ci + 1))
                    for ki in range(3):
                        nc.tensor.matmul(
                            ps7[:, sl],
                            w7xT[ki][64 * h:64 * h + 64, 128 * og:128 * (og + 1)],
                            xb[ki][64 * h:64 * h + 64, sl],
                            start=(ki == 0), stop=(ki == 2))
                nc.scalar.activation(
                    u7[:, HB * h:HB * (h + 1)], ps7[:, 0:HB],
                    AF.Identity, bias=bias7[:, og:og + 1],
                    accum_out=hs7[:, h:h + 1])
            s, b = head_norm_stats(u7, hs7, 2)
            t = hb_pool.tile([128, N], BF16, tag=f"h7b{og}")
            nc.scalar.activation(t[:], u7[:], AF.Prelu, bias=b[:, :], scale=s[:, :], alpha=0.2)
            h7b.append(t)

        h8b = []
        for og in range(2):
            u8 = hu_pool.tile([128, N], F32, tag="uh", name="u8")
            hs8 = small("hs_seg", (128, 2))
            for h in range(2):
                ps8 = pp.tile([128, HB], F32, tag="pp")
                for ci in range(4):
                    sl = slice(HB * h + 512 * ci, HB * h + 512 * (ci + 1))
                    for ki in range(4):
                        nc.tensor.matmul(
                            ps8[:, 512 * ci:512 * (ci + 1)],
                            w8T[ki][:, 128 * og:128 * (og + 1)],
                            h7b[ki][:, sl],
                            start=(ki == 0), stop=(ki == 3))
                nc.scalar.activation(
                    u8[:, HB * h:HB * (h + 1)], ps8[:, 0:HB], AF.Copy,
                    accum_out=hs8[:, h:h + 1])
            s, b = head_norm_stats(u8, hs8, 2)
            t = hb_pool.tile([128, N], BF16, tag=f"h8b{og}")
            nc.scalar.activation(t[:], u8[:], AF.Prelu, bias=b[:, :], scale=s[:, :], alpha=0.2)
            h8b.append(t)

        o2 = hu_pool.tile([2, N], F32, tag="uh", name="o2")
        for ci in range(8):
            sl = slice(512 * ci, 512 * (ci + 1))
            ps9 = pp.tile([128, HB], F32, tag="pp")
            for ki in range(2):
                nc.tensor.matmul(
                    ps9[0:2, 0:512],
                    w9T[ki][:], h8b[ki][:, sl],
                    start=(ki == 0), stop=(ki == 1))
            nc.scalar.copy(o2[:, sl], ps9[0:2, 0:512])

        ost = hb_pool.tile([128, 64], F32, tag="ost")
        pst = pp.tile([128, HB], F32, tag="pp")
        for t in range(32):
            nc.tensor.transpose(
                pst[:, 2 * t:2 * (t + 1)], o2[:, 128 * t:128 * (t + 1)], id2[:])
        nc.scalar.copy(ost[:], pst[:, 0:64])
        nc.sync.dma_start(
            out_d.rearrange("(t p) c -> p t c", p=128),
            ost[:].rearrange("p (t c) -> p t c", c=2))

    nc.finalize()
    return nc


def _shared_inputs(ws):
    import ml_dtypes
    w1, w2, w3, w4, w5, w6, w7, w8, w9 = ws
    f32 = np.float32
    bf16 = ml_dtypes.bfloat16
    d = {}
    for i, w in [(1, w1), (3, w3), (5, w5)]:
        C = w.shape[1] // 2
        d[f"waT{i}"] = np.ascontiguousarray(w[:, :C].T.astype(f32))
        d[f"wbT{i}"] = np.ascontiguousarray((w[:, C:] - w[:, :C]).T.astype(f32))
    d["w2T"] = np.ascontiguousarray(w2.T.astype(f32))
    d["w4T"] = np.ascontiguousarray(w4.T.astype(f32))
    w6t = w6.T.astype(bf16); w7gt = w7[:, :1024].T.astype(bf16)
    w7xt = w7[:, 1024:].T.astype(bf16); w8t = w8.T.astype(bf16)
    w9t = w9.T.astype(bf16)
    for k in range(3):
        d[f"w6T_{k}"] = np.ascontiguousarray(w6t[64 * k:64 * (k + 1)])
        d[f"w7xT_{k}"] = np.ascontiguousarray(w7xt[64 * k:64 * (k + 1)])
    for k in range(8):
        d[f"w7gT_{k}"] = np.ascontiguousarray(w7gt[128 * k:128 * (k + 1)])
    for k in range(4):
        d[f"w8T_{k}"] = np.ascontiguousarray(w8t[128 * k:128 * (k + 1)])
    for k in range(2):
        d[f"w9T_{k}"] = np.ascontiguousarray(w9t[128 * k:128 * (k + 1)])
    d["iota256"] = np.broadcast_to(
        np.arange(CHUNK, dtype=np.uint32)[None, :], (128, CHUNK)).copy()
    d["maskc"] = np.full((128, 1), 0xFFFFFF00, dtype=np.uint32)
    d["id2"] = np.eye(2, dtype=f32)
    return d


def _run(inputs, want_debug=False):
    from concourse.bass_utils import run_bass_kernel_spmd

    if "nc" not in _CACHE:
        _CACHE["nc"] = _build_program()
    nc = _CACHE["nc"]

    x = np.asarray(inputs["x"], dtype=np.float32)
    ws = [np.asarray(inputs[f"w{i}"], dtype=np.float32) for i in range(1, 10)]
    shared = _shared_inputs(ws)
    in_maps = []
    for c in range(8):
        s, half = c >> 1, c & 1
        m = dict(shared)
        xtf = np.ascontiguousarray(x[s].T.astype(np.float32))
        m["xtf"] = xtf
        m["xt"] = np.ascontiguousarray(xtf[:, HB * half:HB * (half + 1)])
        in_maps.append(m)
    res = run_bass_kernel_spmd(nc, in_maps, list(range(8)))
    out = np.stack([res.results[2 * s]["out"] for s in range(4)])
    if want_debug:
        return out, [res.results[c] for c in range(4)]
    return out


def kernel(**inputs):
    return _run(inputs)
